# revision 1
# baseline (speedup 1.0000x reference)
"""Trainium2 Bass kernel for a 2-layer autoregressive LSTM (teacher-forced)
with zoneout (eval mode), conditioning input, and output projection.

Strategy (8 NeuronCores, one TRN2 chip):
  - Model-parallel over the 4*H=4096 gate dimension: core k owns hidden units
    [k*128, (k+1)*128) of each gate (i, f, o, g) for BOTH layers, full batch.
  - Per timestep each core computes its 512 gate rows with gate-stationary
    [128,128] matmul tiles (rhs = h^T [128, 32]), the LSTM cell elementwise on
    [128, 32] tiles, then all cores AllGather their 128-unit h slices so
    everyone has the full h for the next step.
  - Input-side products are hoisted off the serial chain and batched over
    L-step chunks: U0 = xin @ W_ih0^T (xin known ahead of time),
    U1 = h0 @ W_ih1^T (lagging layer 1 by LAG steps), y = h1 @ proj^T
    (proj split 10 output rows per core).  U terms enter the per-step PSUM
    accumulation through an identity-weight matmul; biases ride along as a
    constant-one feature row.
"""

import numpy as np

import concourse.bass as bass
import concourse.bacc as bacc
import concourse.tile as tile
from concourse import bass_utils, mybir

AF = mybir.ActivationFunctionType
ALU = mybir.AluOpType

# Problem constants
B, D, T_FULL, C, H = 32, 80, 1000, 512, 1024
ZONEOUT = 0.1

# Kernel layout constants
P = 128          # partitions
NC = 8           # cores
HU = H // NC     # hidden units per core = 128
MT = 4           # gate m-tiles per core (i, f, o, g)
KIN = 5          # xin contraction chunks (592+bias padded to 640 = 5*128)
KH = H // P      # h contraction chunks = 8
L = 16           # time-chunk length for the batched GEMMs
LAG = L + 4      # layer-1 lag behind layer 0
PJ = D // NC     # proj rows per core = 10
PJP = 16         # padded proj rows per core

BF16 = mybir.dt.bfloat16
F32 = mybir.dt.float32
NP_BF16 = mybir.dt.np(BF16)

RG = [list(range(NC))]


def _chunks(T):
    n = (T + L - 1) // L
    return [(c, min(L, T - c * L)) for c in range(n)]


def build_nc(T):
    """Build the SPMD Bass program for sequence length T."""
    TB = T * B
    nc = bacc.Bacc(
        "TRN2",
        target_bir_lowering=False,
        debug=False,
        enable_asserts=False,
        num_devices=NC,
    )

    # ---- I/O ----
    xinT_d = nc.dram_tensor("xinT", [P, KIN, TB], BF16, kind="ExternalInput")
    w0T_d = nc.dram_tensor("w0T", [P, KIN, MT, P], BF16, kind="ExternalInput")
    wh0T_d = nc.dram_tensor("wh0T", [P, KH, MT, P], BF16, kind="ExternalInput")
    w1T_d = nc.dram_tensor("w1T", [P, KH + 1, MT, P], BF16, kind="ExternalInput")
    wh1T_d = nc.dram_tensor("wh1T", [P, KH, MT, P], BF16, kind="ExternalInput")
    pjT_d = nc.dram_tensor("pjT", [P, KH + 1, PJP], BF16, kind="ExternalInput")
    id_d = nc.dram_tensor("ident", [P, P], BF16, kind="ExternalInput")
    y_d = nc.dram_tensor("y_out", [PJP, TB], F32, kind="ExternalOutput")

    ch = _chunks(T)
    nch = len(ch)
    # chunk emission schedules: iteration t -> chunk index
    u0_at = {(c - 1) * L: c for c, _ in ch if c >= 1}
    u1_at = {c * L + lc: c for c, lc in ch}
    pj_at = {c * L + lc + LAG: c for c, lc in ch}

    with tile.TileContext(nc) as tc:
        with (
            tc.tile_pool(name="const", bufs=1) as cp,
            tc.tile_pool(name="work", bufs=3) as wp,
            tc.tile_pool(name="dram", bufs=3, space="DRAM") as dp,
            tc.tile_pool(name="ps0", bufs=2, space="PSUM") as ps0p,
            tc.tile_pool(name="ps1", bufs=2, space="PSUM") as ps1p,
            tc.tile_pool(name="psu", bufs=2, space="PSUM") as psup,
            tc.tile_pool(name="psp", bufs=1, space="PSUM") as pspp,
        ):
            # resident tiles
            w0_sb = cp.tile([P, KIN, MT, P], BF16)
            wh0_sb = cp.tile([P, KH, MT, P], BF16)
            w1_sb = cp.tile([P, KH + 1, MT, P], BF16)
            wh1_sb = cp.tile([P, KH, MT, P], BF16)
            pj_sb = cp.tile([P, KH + 1, PJP], BF16)
            id_sb = cp.tile([P, P], BF16)
            h0_hist = cp.tile([P, KH + 1, 2 * L, B], BF16)
            h1_hist = cp.tile([P, KH + 1, 2 * L, B], BF16)
            U0_sb = cp.tile([P, 2, MT, L * B], BF16)
            U1_sb = cp.tile([P, 2, MT, L * B], BF16)
            c0_t = cp.tile([P, B], F32)
            h0_t = cp.tile([P, B], F32)
            c1_t = cp.tile([P, B], F32)
            h1_t = cp.tile([P, B], F32)

            nc.sync.dma_start(w0_sb[:], w0T_d[:])
            nc.sync.dma_start(wh0_sb[:], wh0T_d[:])
            nc.sync.dma_start(w1_sb[:], w1T_d[:])
            nc.sync.dma_start(wh1_sb[:], wh1T_d[:])
            nc.sync.dma_start(pj_sb[:], pjT_d[:])
            nc.sync.dma_start(id_sb[:], id_d[:])

            nc.vector.memset(h0_hist[:], 0.0)
            nc.vector.memset(h1_hist[:], 0.0)
            nc.vector.memset(h0_hist[:, KH, :, :], 1.0)  # bias ones-row block
            nc.vector.memset(h1_hist[:, KH, :, :], 1.0)
            nc.vector.memset(c0_t[:], 0.0)
            nc.vector.memset(h0_t[:], 0.0)
            nc.vector.memset(c1_t[:], 0.0)
            nc.vector.memset(h1_t[:], 0.0)

            def emit_u0(c):
                lc = ch[c][1]
                nco = lc * B
                xin_t = wp.tile([P, KIN, L * B], BF16, tag="xin")
                nc.sync.dma_start(
                    xin_t[:, :, :nco], xinT_d[:, :, c * L * B : c * L * B + nco]
                )
                for m in range(MT):
                    pt = psup.tile([P, L * B], F32, tag="psu")
                    for k in range(KIN):
                        nc.tensor.matmul(
                            pt[:, :nco],
                            w0_sb[:, k, m, :],
                            xin_t[:, k, :nco],
                            start=(k == 0),
                            stop=(k == KIN - 1),
                        )
                    nc.vector.tensor_copy(U0_sb[:, c % 2, m, :nco], pt[:, :nco])

            def emit_u1(c):
                lc = ch[c][1]
                nco = lc * B
                half = (c % 2) * L
                for m in range(MT):
                    pt = psup.tile([P, L * B], F32, tag="psu")
                    for k in range(KH + 1):
                        rhs = h0_hist[:, k, half : half + lc, :].rearrange(
                            "p l b -> p (l b)"
                        )
                        nc.tensor.matmul(
                            pt[:, :nco],
                            w1_sb[:, k, m, :],
                            rhs,
                            start=(k == 0),
                            stop=(k == KH),
                        )
                    nc.vector.tensor_copy(U1_sb[:, c % 2, m, :nco], pt[:, :nco])

            def emit_proj(c):
                lc = ch[c][1]
                nco = lc * B
                half = (c % 2) * L
                pt = pspp.tile([PJP, L * B], F32, tag="psp")
                for k in range(KH + 1):
                    rhs = h1_hist[:, k, half : half + lc, :].rearrange(
                        "p l b -> p (l b)"
                    )
                    nc.tensor.matmul(
                        pt[:, :nco],
                        pj_sb[:, k, :],
                        rhs,
                        start=(k == 0),
                        stop=(k == KH),
                    )
                y_t = wp.tile([PJP, L * B], F32, tag="ysb")
                nc.scalar.copy(y_t[:, :nco], pt[:, :nco])
                nc.sync.dma_start(y_d[:, c * L * B : c * L * B + nco], y_t[:, :nco])

            def cell(ell, t, send_t):
                hist = h0_hist if ell == 0 else h1_hist
                W = wh0_sb if ell == 0 else wh1_sb
                U = U0_sb if ell == 0 else U1_sb
                cst = c0_t if ell == 0 else c1_t
                hst = h0_t if ell == 0 else h1_t
                pool = ps0p if ell == 0 else ps1p
                ps = pool.tile([P, MT * B], F32, tag=f"ps{ell}")
                slot = (t - 1) % (2 * L)
                ci, si = t // L, t % L
                for m in range(MT):
                    o = ps[:, m * B : (m + 1) * B]
                    for k in range(KH):
                        nc.tensor.matmul(
                            o, W[:, k, m, :], hist[:, k, slot, :],
                            start=(k == 0), stop=False,
                        )
                    nc.tensor.matmul(
                        o, id_sb[:],
                        U[:, ci % 2, m, si * B : (si + 1) * B],
                        start=False, stop=True,
                    )
                S = wp.tile([P, 3 * B], F32, tag=f"S{ell}")
                nc.scalar.activation(S[:], ps[:, 0 : 3 * B], AF.Sigmoid)
                Tg = wp.tile([P, B], F32, tag=f"Tg{ell}")
                nc.scalar.activation(Tg[:], ps[:, 3 * B : 4 * B], AF.Tanh)
                c01 = wp.tile([P, B], F32, tag=f"c01{ell}")
                nc.vector.tensor_scalar_mul(c01[:], cst[:], 0.1)
                h01 = wp.tile([P, B], F32, tag=f"h01{ell}")
                nc.vector.tensor_scalar_mul(h01[:], hst[:], 0.1)
                So9 = wp.tile([P, B], F32, tag=f"So9{ell}")
                nc.vector.tensor_scalar_mul(So9[:], S[:, 2 * B : 3 * B], 0.9)
                R = wp.tile([P, B], F32, tag=f"R{ell}")
                nc.vector.tensor_mul(R[:], S[:, B : 2 * B], cst[:])
                Pi = wp.tile([P, B], F32, tag=f"Pi{ell}")
                nc.vector.tensor_mul(Pi[:], S[:, 0:B], Tg[:])
                cn = wp.tile([P, B], F32, tag=f"cn{ell}")
                nc.vector.tensor_add(cn[:], R[:], Pi[:])
                # c <- 0.9*c_new + 0.1*c_old
                nc.vector.scalar_tensor_tensor(
                    cst[:], cn[:], 0.9, c01[:], op0=ALU.mult, op1=ALU.add
                )
                Tc = wp.tile([P, B], F32, tag=f"Tc{ell}")
                nc.scalar.activation(Tc[:], cn[:], AF.Tanh)
                Hz = wp.tile([P, B], F32, tag=f"Hz{ell}")
                nc.vector.tensor_mul(Hz[:], So9[:], Tc[:])
                # h <- 0.9*o*tanh(c_new) + 0.1*h_old
                nc.vector.tensor_add(hst[:], Hz[:], h01[:])
                nc.scalar.copy(send_t[:, ell * B : (ell + 1) * B], hst[:])

            emit_u0(0)

            for t in range(T + LAG):
                send_t = wp.tile([P, 2 * B], BF16, tag="send")
                if t < T:
                    cell(0, t, send_t)
                else:
                    nc.vector.memset(send_t[:, 0:B], 0.0)
                tau = t - LAG
                if tau >= 0:
                    cell(1, tau, send_t)
                else:
                    nc.vector.memset(send_t[:, B : 2 * B], 0.0)

                agi = dp.tile([P, 2 * B], BF16, tag="agi")
                ago = dp.tile([NC * P, 2 * B], BF16, tag="ago")
                nc.sync.dma_start(agi[:], send_t[:])
                nc.gpsimd.collective_compute(
                    "AllGather",
                    ALU.bypass,
                    replica_groups=RG,
                    ins=[agi.opt()],
                    outs=[ago.opt()],
                )
                agov = ago[:].rearrange("(k p) b -> p k b", p=P)
                if t < T:
                    nc.sync.dma_start(
                        h0_hist[:, 0:KH, t % (2 * L), :], agov[:, :, 0:B]
                    )
                if tau >= 0:
                    nc.sync.dma_start(
                        h1_hist[:, 0:KH, tau % (2 * L), :], agov[:, :, B : 2 * B]
                    )

                if t in u0_at:
                    emit_u0(u0_at[t])
                if t in u1_at:
                    emit_u1(u1_at[t])
                if t in pj_at:
                    emit_proj(pj_at[t])

            # chunks scheduled past the last iteration
            for t_late in sorted(pj_at):
                if t_late >= T + LAG:
                    emit_proj(pj_at[t_late])

    nc.compile()
    return nc


# ---------------- host-side data prep ----------------

def _gate_rows(k):
    u = np.arange(k * HU, (k + 1) * HU)
    return np.concatenate([u, H + u, 3 * H + u, 2 * H + u])  # i, f, o, g


def _lhsT_blocks(w, nk, mt=MT):
    """w: [mt*P, nk*P] (already row-sliced/ordered) -> [P, nk, mt, P] lhsT tiles."""
    a = w.reshape(mt, P, nk, P)  # [m, j, k, p]
    return np.ascontiguousarray(a.transpose(3, 2, 0, 1))  # [p, k, m, j]


def prep_inputs(inputs, T):
    x = np.asarray(inputs["x"], np.float32)[:, :, :T]
    cond = np.asarray(inputs["cond"], np.float32)[:, :, :T]
    w_ih0 = np.asarray(inputs["w_ih0"], np.float32)
    w_hh0 = np.asarray(inputs["w_hh0"], np.float32)
    b0 = np.asarray(inputs["b_ih0"], np.float32) + np.asarray(inputs["b_hh0"], np.float32)
    w_ih1 = np.asarray(inputs["w_ih1"], np.float32)
    w_hh1 = np.asarray(inputs["w_hh1"], np.float32)
    b1 = np.asarray(inputs["b_ih1"], np.float32) + np.asarray(inputs["b_hh1"], np.float32)
    proj_w = np.asarray(inputs["proj_w"], np.float32)
    proj_b = np.asarray(inputs["proj_b"], np.float32)

    TB = T * B
    in0 = D + C
    xs = np.concatenate([np.zeros((B, D, 1), np.float32), x[:, :, : T - 1]], axis=2)
    xin = np.concatenate([xs, cond], axis=1)  # [B, 592, T]
    xin_pad = np.zeros((B, KIN * P, T), np.float32)
    xin_pad[:, :in0] = xin
    xin_pad[:, in0] = 1.0  # bias feature
    # [feat, T, B] -> [feat, TB] with col index t*B+b
    xinT = np.ascontiguousarray(xin_pad.transpose(1, 2, 0)).reshape(KIN * P, TB)
    xinT = np.ascontiguousarray(
        xinT.reshape(KIN, P, TB).transpose(1, 0, 2)
    ).astype(NP_BF16)

    w_ih0_pad = np.zeros((4 * H, KIN * P), np.float32)
    w_ih0_pad[:, :in0] = w_ih0
    w_ih0_pad[:, in0] = b0

    ident = np.eye(P, dtype=NP_BF16)

    in_maps = []
    for k in range(NC):
        r = _gate_rows(k)
        w0T = _lhsT_blocks(w_ih0_pad[r], KIN).astype(NP_BF16)
        wh0T = _lhsT_blocks(w_hh0[r], KH).astype(NP_BF16)
        w1_ext = np.zeros((MT * P, (KH + 1) * P), np.float32)
        w1_ext[:, : KH * P] = w_ih1[r]
        w1_ext[:, KH * P] = b1[r]  # ones-row bias block (row 0 of chunk KH)
        w1T = _lhsT_blocks(w1_ext, KH + 1).astype(NP_BF16)
        wh1T = _lhsT_blocks(w_hh1[r], KH).astype(NP_BF16)
        pjT = np.zeros((P, KH + 1, PJP), np.float32)
        rows = np.arange(k * PJ, (k + 1) * PJ)
        for kk in range(KH):
            pjT[:, kk, :PJ] = proj_w[rows, kk * P : (kk + 1) * P].T
        pjT[0, KH, :PJ] = proj_b[rows]
        in_maps.append(
            {
                "xinT": xinT,
                "w0T": w0T,
                "wh0T": wh0T,
                "w1T": w1T,
                "wh1T": wh1T,
                "pjT": pjT.astype(NP_BF16),
                "ident": ident,
            }
        )
    return in_maps


def assemble(results, x_lengths, T):
    y = np.concatenate([r["y_out"][:PJ] for r in results], axis=0)  # [80, TB]
    y = y.reshape(D, T, B).transpose(2, 0, 1)  # [B, D, T]
    lens = np.asarray(x_lengths).astype(np.int64)
    mask = (np.arange(T)[None, :] < lens[:, None]).astype(np.float32)
    return np.ascontiguousarray(y * mask[:, None, :])


_NC_CACHE = {}


def run(inputs, T=T_FULL, trace=False, **kw):
    if T not in _NC_CACHE:
        _NC_CACHE[T] = build_nc(T)
    nc = _NC_CACHE[T]
    in_maps = prep_inputs(inputs, T)
    res = bass_utils.run_bass_kernel_spmd(
        nc, in_maps, core_ids=list(range(NC)), trace=trace, **kw
    )
    out = assemble(res.results, inputs["x_lengths"], T)
    return out, res


def kernel(**inputs) -> np.ndarray:
    out, _ = run(inputs, T=T_FULL)
    return out



# revision 2
# speedup vs baseline: 2.4536x; 2.4536x over previous
"""Trainium2 Bass kernel for the 2-layer autoregressive LSTM (teacher-forced).

v2 -> v3 changes, driven by cost-model + HW ablation findings:
  - Sigmoid-only activations: tanh(x) = 2*sigmoid(2x) - 1, with the 2x folded
    into the g-gate weight rows host-side.  One activation per cell covers all
    four gates; no ACT table reloads (1.28us each) ever.
  - Combined history: one SBUF tile holds [h0(t), h1(t-LAG)] per ring slot
    (slot = t mod 2L, LAG = 2L), so ONE history DMA per step instead of two.
  - Chunked GEMMs read strided 3D rhs views of the combined history.
  - Element-wise cell trimmed to 2 ACT + 9 DVE ops via two-stage
    tensor_scalar affines.
"""

import numpy as np

import concourse.bass as bass
import concourse.bacc as bacc
import concourse.tile as tile
from concourse import bass_utils, mybir

AF = mybir.ActivationFunctionType
ALU = mybir.AluOpType

# Problem constants
B, D, T_FULL, C, H = 32, 80, 1000, 512, 1024
ZONEOUT = 0.1

# Kernel layout constants
P = 128
NC = 8
HU = H // NC
MT = 4
KIN = 5
KH = H // P      # 8
L = 16
LAG = 2 * L      # layer-1 lags a full ring behind layer 0
PJ = D // NC
PJP = 16

BF16 = mybir.dt.bfloat16
F32 = mybir.dt.float32
NP_BF16 = mybir.dt.np(BF16)

RG = [list(range(NC))]


def _chunks(T):
    n = (T + L - 1) // L
    return [(c, min(L, T - c * L)) for c in range(n)]


def build_nc(T):
    TB = T * B
    nc = bacc.Bacc(
        "TRN2",
        target_bir_lowering=False,
        debug=False,
        enable_asserts=False,
        num_devices=NC,
    )

    xinT_d = nc.dram_tensor("xinT", [P, KIN, TB], BF16, kind="ExternalInput")
    w0T_d = nc.dram_tensor("w0T", [P, KIN, MT, P], BF16, kind="ExternalInput")
    wh0T_d = nc.dram_tensor("wh0T", [P, KH, MT, P], BF16, kind="ExternalInput")
    w1T_d = nc.dram_tensor("w1T", [P, KH + 1, MT, P], BF16, kind="ExternalInput")
    wh1T_d = nc.dram_tensor("wh1T", [P, KH, MT, P], BF16, kind="ExternalInput")
    pjT_d = nc.dram_tensor("pjT", [P, KH + 1, PJP], BF16, kind="ExternalInput")
    id_d = nc.dram_tensor("ident", [P, P], BF16, kind="ExternalInput")
    y_d = nc.dram_tensor("y_out", [PJP, TB], F32, kind="ExternalOutput")

    ch = _chunks(T)
    u0_at = {(c - 1) * L: c for c, _ in ch if c >= 1}
    u1_at = {c * L + lc: c for c, lc in ch}
    pj_at = {c * L + lc + LAG: c for c, lc in ch}

    with tile.TileContext(nc) as tc:
        with (
            tc.tile_pool(name="const", bufs=1) as cp,
            tc.tile_pool(name="dram", bufs=1, space="DRAM") as dp,
            tc.tile_pool(name="psc", bufs=1, space="PSUM") as pscp,
            tc.tile_pool(name="psu", bufs=1, space="PSUM") as psup,
        ):
            w0_sb = cp.tile([P, KIN, MT, P], BF16)
            wh0_sb = cp.tile([P, KH, MT, P], BF16)
            w1_sb = cp.tile([P, KH + 1, MT, P], BF16)
            wh1_sb = cp.tile([P, KH, MT, P], BF16)
            pj_sb = cp.tile([P, KH + 1, PJP], BF16)
            id_sb = cp.tile([P, P], BF16)
            # combined ring history: slot s holds [h0(s), h1(s-LAG)] slices
            hist = cp.tile([P, KH, 2 * L, 2 * B], BF16)
            ones_sb = cp.tile([P, L, B], BF16)
            U0_sb = cp.tile([P, 2, MT, L * B], BF16)
            U1_sb = cp.tile([P, 2, MT, L * B], BF16)
            xin_t = cp.tile([P, 2, KIN, L * B], BF16)
            y_t = cp.tile([PJP, 2, L * B], F32)

            c0_t = cp.tile([P, B], F32)
            h0_t = cp.tile([P, B], F32)
            c1_t = cp.tile([P, B], F32)
            h1_t = cp.tile([P, B], F32)

            send_t = cp.tile([P, 2, 2 * B], BF16)
            S_t = cp.tile([P, 2, 2, MT * B], F32)
            tg_t = cp.tile([P, 2, 2, B], F32)
            cn_t = cp.tile([P, 2, 2, B], F32)
            c01_t = cp.tile([P, 2, 2, B], F32)
            h01_t = cp.tile([P, 2, 2, B], F32)
            sc_t = cp.tile([P, 2, 2, B], F32)
            Hz_t = cp.tile([P, 2, 2, B], F32)
            R_t = cp.tile([P, 2, 2, B], F32)
            Pi_t = cp.tile([P, 2, 2, B], F32)

            ps_cell = pscp.tile([P, 4, MT * B], F32)
            ps_u = psup.tile([P, 2, L * B], F32)
            ps_pj = psup.tile([PJP, 2, L * B], F32)

            agi0 = dp.tile([P, 2 * B], BF16, tag="agi0")
            agi1 = dp.tile([P, 2 * B], BF16, tag="agi1")
            ago0 = dp.tile([NC * P, 2 * B], BF16, tag="ago0")
            ago1 = dp.tile([NC * P, 2 * B], BF16, tag="ago1")
            agi = [agi0, agi1]
            ago = [ago0, ago1]

            nc.sync.dma_start(w0_sb[:], w0T_d[:])
            nc.sync.dma_start(wh0_sb[:], wh0T_d[:])
            nc.sync.dma_start(w1_sb[:], w1T_d[:])
            nc.sync.dma_start(wh1_sb[:], wh1T_d[:])
            nc.sync.dma_start(pj_sb[:], pjT_d[:])
            nc.sync.dma_start(id_sb[:], id_d[:])

            nc.vector.memset(hist[:], 0.0)
            nc.vector.memset(ones_sb[:], 1.0)
            nc.vector.memset(c0_t[:], 0.0)
            nc.vector.memset(h0_t[:], 0.0)
            nc.vector.memset(c1_t[:], 0.0)
            nc.vector.memset(h1_t[:], 0.0)
            nc.vector.memset(send_t[:], 0.0)

            def h0_rhs(k, c, lc):
                # h0 over chunk-c timesteps: slots c*L .. c*L+lc-1, layer-0 half
                half = (c * L) % (2 * L)
                return hist[:, k, half : half + lc, 0:B]

            def h1_rhs(k, c, lc):
                # h1(tau) lives in slot tau+LAG = tau (mod 2L), layer-1 half
                half = (c * L) % (2 * L)
                return hist[:, k, half : half + lc, B : 2 * B]

            def emit_u0(c):
                lc = ch[c][1]
                nco = lc * B
                xt = xin_t[:, c % 2, :, :]
                nc.sync.dma_start(
                    xt[:, :, :nco], xinT_d[:, :, c * L * B : c * L * B + nco]
                )
                for m in range(MT):
                    pt = ps_u[:, c % 2, :]
                    for k in range(KIN):
                        nc.tensor.matmul(
                            pt[:, :nco],
                            w0_sb[:, k, m, :],
                            xt[:, k, :nco],
                            start=(k == 0),
                            stop=(k == KIN - 1),
                        )
                    nc.vector.tensor_copy(U0_sb[:, c % 2, m, :nco], pt[:, :nco])

            def emit_u1(c):
                lc = ch[c][1]
                nco = lc * B
                for m in range(MT):
                    pt = ps_u[:, c % 2, :]
                    for k in range(KH + 1):
                        if k < KH:
                            rhs = h0_rhs(k, c, lc)
                        else:
                            rhs = ones_sb[:, 0:lc, :]
                        nc.tensor.matmul(
                            pt[:, :nco],
                            w1_sb[:, k, m, :],
                            rhs,
                            start=(k == 0),
                            stop=(k == KH),
                        )
                    nc.vector.tensor_copy(U1_sb[:, c % 2, m, :nco], pt[:, :nco])

            def emit_proj(c):
                lc = ch[c][1]
                nco = lc * B
                pt = ps_pj[:, c % 2, :]
                for k in range(KH + 1):
                    if k < KH:
                        rhs = h1_rhs(k, c, lc)
                    else:
                        rhs = ones_sb[:, 0:lc, :]
                    nc.tensor.matmul(
                        pt[:, :nco],
                        pj_sb[:, k, :],
                        rhs,
                        start=(k == 0),
                        stop=(k == KH),
                    )
                yt = y_t[:, c % 2, :]
                nc.scalar.copy(yt[:, :nco], pt[:, :nco])
                nc.sync.dma_start(y_d[:, c * L * B : c * L * B + nco], yt[:, :nco])

            def cell(ell, t, par):
                off = 0 if ell == 0 else B
                W = wh0_sb if ell == 0 else wh1_sb
                U = U0_sb if ell == 0 else U1_sb
                cst = c0_t if ell == 0 else c1_t
                hst = h0_t if ell == 0 else h1_t
                ps = ps_cell[:, ell * 2 + par, :]
                slot = (t - 1) % (2 * L)
                ci, si = t // L, t % L
                for m in range(MT):
                    o = ps[:, m * B : (m + 1) * B]
                    for k in range(KH):
                        nc.tensor.matmul(
                            o, W[:, k, m, :], hist[:, k, slot, off : off + B],
                            start=(k == 0), stop=False,
                        )
                    nc.tensor.matmul(
                        o, id_sb[:],
                        U[:, ci % 2, m, si * B : (si + 1) * B],
                        start=False, stop=True,
                    )
                S = S_t[:, par, ell, :]
                tg = tg_t[:, par, ell, :]
                cn = cn_t[:, par, ell, :]
                c01 = c01_t[:, par, ell, :]
                h01 = h01_t[:, par, ell, :]
                sc = sc_t[:, par, ell, :]
                Hz = Hz_t[:, par, ell, :]
                R = R_t[:, par, ell, :]
                Pi = Pi_t[:, par, ell, :]
                # gates: [i, f, o, 2g] -> sigmoid of all four at once
                nc.scalar.activation(S[:], ps[:], AF.Sigmoid)
                # tanh(g) = 2*sigmoid(2g) - 1
                nc.vector.tensor_scalar(
                    tg[:], S[:, 3 * B : 4 * B], 2.0, 1.0,
                    op0=ALU.mult, op1=ALU.subtract,
                )
                nc.vector.tensor_mul(Pi[:], S[:, 0:B], tg[:])
                nc.vector.tensor_mul(R[:], S[:, B : 2 * B], cst[:])
                nc.vector.tensor_add(cn[:], R[:], Pi[:])
                nc.vector.tensor_scalar_mul(c01[:], cst[:], ZONEOUT)
                nc.vector.scalar_tensor_tensor(
                    cst[:], cn[:], 1.0 - ZONEOUT, c01[:], op0=ALU.mult, op1=ALU.add
                )
                # tanh(c_new) = 2*sigmoid(2*c_new) - 1, folded with 0.9:
                # 0.9*tanh(cn) = 1.8*sigmoid(2*cn) - 0.9
                nc.scalar.activation(sc[:], cn[:], AF.Sigmoid, scale=2.0)
                nc.vector.tensor_scalar(
                    Hz[:], sc[:], 2.0 * (1.0 - ZONEOUT), 1.0 - ZONEOUT,
                    op0=ALU.mult, op1=ALU.subtract,
                )
                nc.vector.tensor_scalar_mul(h01[:], hst[:], ZONEOUT)
                nc.vector.tensor_mul(Hz[:], Hz[:], S[:, 2 * B : 3 * B])
                nc.vector.tensor_add(hst[:], Hz[:], h01[:])
                nc.scalar.copy(send_t[:, par, off : off + B], hst[:])

            emit_u0(0)

            for t in range(T + LAG):
                par = t % 2
                if t < T:
                    cell(0, t, par)
                tau = t - LAG
                if tau >= 0:
                    cell(1, tau, par)

                nc.gpsimd.dma_start(agi[par][:], send_t[:, par, :])
                nc.gpsimd.collective_compute(
                    "AllGather",
                    ALU.bypass,
                    replica_groups=RG,
                    ins=[agi[par].opt()],
                    outs=[ago[par].opt()],
                )
                agov = ago[par][:].rearrange("(k p) b -> p k b", p=P)
                nc.gpsimd.dma_start(hist[:, :, t % (2 * L), :], agov)

                if t in u0_at:
                    emit_u0(u0_at[t])
                if t in u1_at:
                    emit_u1(u1_at[t])
                if t in pj_at:
                    emit_proj(pj_at[t])

            for t_late in sorted(pj_at):
                if t_late >= T + LAG:
                    emit_proj(pj_at[t_late])

    nc.compile()
    return nc


# ---------------- host-side data prep ----------------

def _gate_rows(k):
    u = np.arange(k * HU, (k + 1) * HU)
    return np.concatenate([u, H + u, 3 * H + u, 2 * H + u])  # i, f, o, g


def _scale_g(w):
    """Scale the g-gate rows (last P block) by 2 for the sigmoid-only trick."""
    w = w.copy()
    w[3 * P :] *= 2.0
    return w


def _lhsT_blocks(w, nk, mt=MT):
    a = w.reshape(mt, P, nk, P)
    return np.ascontiguousarray(a.transpose(3, 2, 0, 1))


def prep_inputs(inputs, T):
    x = np.asarray(inputs["x"], np.float32)[:, :, :T]
    cond = np.asarray(inputs["cond"], np.float32)[:, :, :T]
    w_ih0 = np.asarray(inputs["w_ih0"], np.float32)
    w_hh0 = np.asarray(inputs["w_hh0"], np.float32)
    b0 = np.asarray(inputs["b_ih0"], np.float32) + np.asarray(inputs["b_hh0"], np.float32)
    w_ih1 = np.asarray(inputs["w_ih1"], np.float32)
    w_hh1 = np.asarray(inputs["w_hh1"], np.float32)
    b1 = np.asarray(inputs["b_ih1"], np.float32) + np.asarray(inputs["b_hh1"], np.float32)
    proj_w = np.asarray(inputs["proj_w"], np.float32)
    proj_b = np.asarray(inputs["proj_b"], np.float32)

    TB = T * B
    in0 = D + C
    xs = np.concatenate([np.zeros((B, D, 1), np.float32), x[:, :, : T - 1]], axis=2)
    xin = np.concatenate([xs, cond], axis=1)
    xin_pad = np.zeros((B, KIN * P, T), np.float32)
    xin_pad[:, :in0] = xin
    xin_pad[:, in0] = 1.0
    xinT = np.ascontiguousarray(xin_pad.transpose(1, 2, 0)).reshape(KIN * P, TB)
    xinT = np.ascontiguousarray(
        xinT.reshape(KIN, P, TB).transpose(1, 0, 2)
    ).astype(NP_BF16)

    w_ih0_pad = np.zeros((4 * H, KIN * P), np.float32)
    w_ih0_pad[:, :in0] = w_ih0
    w_ih0_pad[:, in0] = b0

    ident = np.eye(P, dtype=NP_BF16)

    in_maps = []
    for k in range(NC):
        r = _gate_rows(k)
        w0T = _lhsT_blocks(_scale_g(w_ih0_pad[r]), KIN).astype(NP_BF16)
        wh0T = _lhsT_blocks(_scale_g(w_hh0[r]), KH).astype(NP_BF16)
        w1_ext = np.zeros((MT * P, (KH + 1) * P), np.float32)
        w1_ext[:, : KH * P] = w_ih1[r]
        w1_ext[:, KH * P] = b1[r]
        w1T = _lhsT_blocks(_scale_g(w1_ext), KH + 1).astype(NP_BF16)
        wh1T = _lhsT_blocks(_scale_g(w_hh1[r]), KH).astype(NP_BF16)
        pjT = np.zeros((P, KH + 1, PJP), np.float32)
        rows = np.arange(k * PJ, (k + 1) * PJ)
        for kk in range(KH):
            pjT[:, kk, :PJ] = proj_w[rows, kk * P : (kk + 1) * P].T
        pjT[0, KH, :PJ] = proj_b[rows]
        in_maps.append(
            {
                "xinT": xinT,
                "w0T": w0T,
                "wh0T": wh0T,
                "w1T": w1T,
                "wh1T": wh1T,
                "pjT": pjT.astype(NP_BF16),
                "ident": ident,
            }
        )
    return in_maps


def assemble(results, x_lengths, T):
    y = np.concatenate([r["y_out"][:PJ] for r in results], axis=0)
    y = y.reshape(D, T, B).transpose(2, 0, 1)
    lens = np.asarray(x_lengths).astype(np.int64)
    mask = (np.arange(T)[None, :] < lens[:, None]).astype(np.float32)
    return np.ascontiguousarray(y * mask[:, None, :])


_NC_CACHE = {}


def run(inputs, T=T_FULL, trace=False, **kw):
    if T not in _NC_CACHE:
        _NC_CACHE[T] = build_nc(T)
    nc = _NC_CACHE[T]
    in_maps = prep_inputs(inputs, T)
    res = bass_utils.run_bass_kernel_spmd(
        nc, in_maps, core_ids=list(range(NC)), trace=trace, **kw
    )
    out = assemble(res.results, inputs["x_lengths"], T)
    return out, res


def kernel(**inputs) -> np.ndarray:
    out, _ = run(inputs, T=T_FULL)
    return out


# revision 3
# speedup vs baseline: 3.2085x; 1.3077x over previous
"""Trainium2 Bass kernel for the 2-layer autoregressive LSTM (teacher-forced).

v2 -> v3 changes, driven by cost-model + HW ablation findings:
  - Sigmoid-only activations: tanh(x) = 2*sigmoid(2x) - 1, with the 2x folded
    into the g-gate weight rows host-side.  One activation per cell covers all
    four gates; no ACT table reloads (1.28us each) ever.
  - Combined history: one SBUF tile holds [h0(t), h1(t-LAG)] per ring slot
    (slot = t mod 2L, LAG = 2L), so ONE history DMA per step instead of two.
  - Chunked GEMMs read strided 3D rhs views of the combined history.
  - Element-wise cell trimmed to 2 ACT + 9 DVE ops via two-stage
    tensor_scalar affines.
"""

import numpy as np

import concourse.bass as bass
import concourse.bacc as bacc
import concourse.tile as tile
from concourse import bass_utils, mybir

AF = mybir.ActivationFunctionType
ALU = mybir.AluOpType

# Problem constants
B, D, T_FULL, C, H = 32, 80, 1000, 512, 1024
ZONEOUT = 0.1

# Kernel layout constants
P = 128
NC = 8
HU = H // NC
MT = 4
KIN = 5
KH = H // P      # 8
L = 16
LAG = 2 * L      # layer-1 lags a full ring behind layer 0
PJ = D // NC
PJP = 16

BF16 = mybir.dt.bfloat16
F32 = mybir.dt.float32
NP_BF16 = mybir.dt.np(BF16)

RG = [list(range(NC))]


def _chunks(T):
    n = (T + L - 1) // L
    return [(c, min(L, T - c * L)) for c in range(n)]


def build_nc(T):
    TB = T * B
    nc = bacc.Bacc(
        "TRN2",
        target_bir_lowering=False,
        debug=False,
        enable_asserts=False,
        num_devices=NC,
    )

    xinT_d = nc.dram_tensor("xinT", [P, KIN, TB], BF16, kind="ExternalInput")
    w0T_d = nc.dram_tensor("w0T", [P, KIN, MT, P], BF16, kind="ExternalInput")
    wh0T_d = nc.dram_tensor("wh0T", [P, KH, MT, P], BF16, kind="ExternalInput")
    w1T_d = nc.dram_tensor("w1T", [P, KH + 1, MT, P], BF16, kind="ExternalInput")
    wh1T_d = nc.dram_tensor("wh1T", [P, KH, MT, P], BF16, kind="ExternalInput")
    pjT_d = nc.dram_tensor("pjT", [P, KH + 1, PJP], BF16, kind="ExternalInput")
    id_d = nc.dram_tensor("ident", [P, P], BF16, kind="ExternalInput")
    y_d = nc.dram_tensor("y_out", [PJP, TB], F32, kind="ExternalOutput")

    ch = _chunks(T)
    u0_at = {(c - 1) * L: c for c, _ in ch if c >= 1}
    u1_at = {c * L + lc: c for c, lc in ch}
    pj_at = {c * L + lc + LAG: c for c, lc in ch}

    with tile.TileContext(nc) as tc:
        with (
            tc.tile_pool(name="const", bufs=1) as cp,
            tc.tile_pool(name="dram", bufs=1, space="DRAM") as dp,
            tc.tile_pool(name="psc", bufs=1, space="PSUM") as pscp,
            tc.tile_pool(name="psu", bufs=1, space="PSUM") as psup,
        ):
            w0_sb = cp.tile([P, KIN, MT, P], BF16)
            wh0_sb = cp.tile([P, KH, MT, P], BF16)
            w1_sb = cp.tile([P, KH + 1, MT, P], BF16)
            wh1_sb = cp.tile([P, KH, MT, P], BF16)
            pj_sb = cp.tile([P, KH + 1, PJP], BF16)
            id_sb = cp.tile([P, P], BF16)
            # combined ring history: slot s holds [h0(s), h1(s-LAG)] slices
            hist = cp.tile([P, KH, 2 * L, 2 * B], BF16)
            ones_sb = cp.tile([P, L, B], BF16)
            U0_sb = cp.tile([P, 2, MT, L * B], BF16)
            U1_sb = cp.tile([P, 2, MT, L * B], BF16)
            xin_t = cp.tile([P, 2, KIN, L * B], BF16)
            y_t = cp.tile([PJP, 2, L * B], F32)

            c0_t = cp.tile([P, B], F32)
            c1_t = cp.tile([P, B], F32)

            send_t = cp.tile([P, 2, 2 * B], BF16)
            S_t = cp.tile([P, 2, 2, MT * B], F32)
            tg_t = cp.tile([P, 2, 2, B], F32)
            cn_t = cp.tile([P, 2, 2, B], F32)
            c01_t = cp.tile([P, 2, 2, B], F32)
            h01_t = cp.tile([P, 2, 2, B], F32)
            sc_t = cp.tile([P, 2, 2, B], F32)
            Hz_t = cp.tile([P, 2, 2, B], F32)
            R_t = cp.tile([P, 2, 2, B], F32)
            Pi_t = cp.tile([P, 2, 2, B], F32)

            ps_cell = pscp.tile([P, 4, MT * B], F32)
            ps_u = psup.tile([P, 2, L * B], F32)
            ps_pj = psup.tile([PJP, 2, L * B], F32)

            agi0 = dp.tile([P, 2 * B], BF16, tag="agi0")
            agi1 = dp.tile([P, 2 * B], BF16, tag="agi1")
            ago0 = dp.tile([NC * P, 2 * B], BF16, tag="ago0")
            ago1 = dp.tile([NC * P, 2 * B], BF16, tag="ago1")
            agi = [agi0, agi1]
            ago = [ago0, ago1]

            nc.sync.dma_start(w0_sb[:], w0T_d[:])
            nc.sync.dma_start(wh0_sb[:], wh0T_d[:])
            nc.sync.dma_start(w1_sb[:], w1T_d[:])
            nc.sync.dma_start(wh1_sb[:], wh1T_d[:])
            nc.sync.dma_start(pj_sb[:], pjT_d[:])
            nc.sync.dma_start(id_sb[:], id_d[:])

            nc.vector.memset(hist[:], 0.0)
            nc.vector.memset(ones_sb[:], 1.0)
            nc.vector.memset(c0_t[:], 0.0)
            nc.vector.memset(c1_t[:], 0.0)
            nc.vector.memset(send_t[:], 0.0)

            def h0_rhs(k, c, lc):
                # h0 over chunk-c timesteps: slots c*L .. c*L+lc-1, layer-0 half
                half = (c * L) % (2 * L)
                return hist[:, k, half : half + lc, 0:B]

            def h1_rhs(k, c, lc):
                # h1(tau) lives in slot tau+LAG = tau (mod 2L), layer-1 half
                half = (c * L) % (2 * L)
                return hist[:, k, half : half + lc, B : 2 * B]

            def emit_u0(c):
                lc = ch[c][1]
                nco = lc * B
                xt = xin_t[:, c % 2, :, :]
                nc.sync.dma_start(
                    xt[:, :, :nco], xinT_d[:, :, c * L * B : c * L * B + nco]
                )
                for m in range(MT):
                    pt = ps_u[:, c % 2, :]
                    for k in range(KIN):
                        nc.tensor.matmul(
                            pt[:, :nco],
                            w0_sb[:, k, m, :],
                            xt[:, k, :nco],
                            start=(k == 0),
                            stop=(k == KIN - 1),
                        )
                    nc.vector.tensor_copy(U0_sb[:, c % 2, m, :nco], pt[:, :nco])

            def emit_u1(c):
                lc = ch[c][1]
                nco = lc * B
                for m in range(MT):
                    pt = ps_u[:, c % 2, :]
                    for k in range(KH + 1):
                        if k < KH:
                            rhs = h0_rhs(k, c, lc)
                        else:
                            rhs = ones_sb[:, 0:lc, :]
                        nc.tensor.matmul(
                            pt[:, :nco],
                            w1_sb[:, k, m, :],
                            rhs,
                            start=(k == 0),
                            stop=(k == KH),
                        )
                    nc.vector.tensor_copy(U1_sb[:, c % 2, m, :nco], pt[:, :nco])

            def emit_proj(c):
                lc = ch[c][1]
                nco = lc * B
                pt = ps_pj[:, c % 2, :]
                for k in range(KH + 1):
                    if k < KH:
                        rhs = h1_rhs(k, c, lc)
                    else:
                        rhs = ones_sb[:, 0:lc, :]
                    nc.tensor.matmul(
                        pt[:, :nco],
                        pj_sb[:, k, :],
                        rhs,
                        start=(k == 0),
                        stop=(k == KH),
                    )
                yt = y_t[:, c % 2, :]
                nc.scalar.copy(yt[:, :nco], pt[:, :nco])
                nc.sync.dma_start(y_d[:, c * L * B : c * L * B + nco], yt[:, :nco])

            def cell(ell, t, par):
                off = 0 if ell == 0 else B
                W = wh0_sb if ell == 0 else wh1_sb
                U = U0_sb if ell == 0 else U1_sb
                cst = c0_t if ell == 0 else c1_t
                ps = ps_cell[:, ell * 2 + par, :]
                slot = (t - 1) % (2 * L)
                ci, si = t // L, t % L
                for m in range(MT):
                    o = ps[:, m * B : (m + 1) * B]
                    for k in range(KH):
                        nc.tensor.matmul(
                            o, W[:, k, m, :], hist[:, k, slot, off : off + B],
                            start=(k == 0), stop=False,
                        )
                    nc.tensor.matmul(
                        o, id_sb[:],
                        U[:, ci % 2, m, si * B : (si + 1) * B],
                        start=False, stop=True,
                    )
                S = S_t[:, par, ell, :]
                tg = tg_t[:, par, ell, :]
                cn = cn_t[:, par, ell, :]
                c01 = c01_t[:, par, ell, :]
                h01 = h01_t[:, par, ell, :]
                sc = sc_t[:, par, ell, :]
                Hz = Hz_t[:, par, ell, :]
                R = R_t[:, par, ell, :]
                Pi = Pi_t[:, par, ell, :]
                # gates: [i, f, o, 2g] -> sigmoid of all four at once
                nc.scalar.activation(S[:], ps[:], AF.Sigmoid)
                # tanh(g) = 2*sigmoid(2g) - 1
                nc.vector.tensor_scalar(
                    tg[:], S[:, 3 * B : 4 * B], 2.0, 1.0,
                    op0=ALU.mult, op1=ALU.subtract,
                )
                nc.vector.tensor_mul(Pi[:], S[:, 0:B], tg[:])
                nc.vector.tensor_mul(R[:], S[:, B : 2 * B], cst[:])
                nc.vector.tensor_add(cn[:], R[:], Pi[:])
                nc.vector.tensor_scalar_mul(c01[:], cst[:], ZONEOUT)
                nc.vector.scalar_tensor_tensor(
                    cst[:], cn[:], 1.0 - ZONEOUT, c01[:], op0=ALU.mult, op1=ALU.add
                )
                # tanh(c_new) = 2*sigmoid(2*c_new) - 1, folded with 0.9:
                # 0.9*tanh(cn) = 1.8*sigmoid(2*cn) - 0.9
                nc.scalar.activation(sc[:], cn[:], AF.Sigmoid, scale=2.0)
                nc.vector.tensor_scalar(
                    Hz[:], sc[:], 2.0 * (1.0 - ZONEOUT), 1.0 - ZONEOUT,
                    op0=ALU.mult, op1=ALU.subtract,
                )
                nc.vector.tensor_scalar_mul(
                    h01[:], send_t[:, 1 - par, off : off + B], ZONEOUT
                )
                nc.vector.tensor_mul(Hz[:], Hz[:], S[:, 2 * B : 3 * B])
                nc.vector.tensor_add(send_t[:, par, off : off + B], Hz[:], h01[:])

            emit_u0(0)

            for t in range(T + LAG):
                par = t % 2
                if t < T:
                    cell(0, t, par)
                tau = t - LAG
                if tau >= 0:
                    cell(1, tau, par)

                nc.gpsimd.dma_start(agi[par][:], send_t[:, par, :])
                nc.gpsimd.collective_compute(
                    "AllGather",
                    ALU.bypass,
                    replica_groups=RG,
                    ins=[agi[par].opt()],
                    outs=[ago[par].opt()],
                )
                agov = ago[par][:].rearrange("(k p) b -> p k b", p=P)
                nc.gpsimd.dma_start(hist[:, :, t % (2 * L), :], agov)

                if t in u0_at:
                    emit_u0(u0_at[t])
                if t in u1_at:
                    emit_u1(u1_at[t])
                if t in pj_at:
                    emit_proj(pj_at[t])

            for t_late in sorted(pj_at):
                if t_late >= T + LAG:
                    emit_proj(pj_at[t_late])

    nc.compile()
    return nc


# ---------------- host-side data prep ----------------

def _gate_rows(k):
    u = np.arange(k * HU, (k + 1) * HU)
    return np.concatenate([u, H + u, 3 * H + u, 2 * H + u])  # i, f, o, g


def _scale_g(w):
    """Scale the g-gate rows (last P block) by 2 for the sigmoid-only trick."""
    w = w.copy()
    w[3 * P :] *= 2.0
    return w


def _lhsT_blocks(w, nk, mt=MT):
    a = w.reshape(mt, P, nk, P)
    return np.ascontiguousarray(a.transpose(3, 2, 0, 1))


def prep_inputs(inputs, T):
    x = np.asarray(inputs["x"], np.float32)[:, :, :T]
    cond = np.asarray(inputs["cond"], np.float32)[:, :, :T]
    w_ih0 = np.asarray(inputs["w_ih0"], np.float32)
    w_hh0 = np.asarray(inputs["w_hh0"], np.float32)
    b0 = np.asarray(inputs["b_ih0"], np.float32) + np.asarray(inputs["b_hh0"], np.float32)
    w_ih1 = np.asarray(inputs["w_ih1"], np.float32)
    w_hh1 = np.asarray(inputs["w_hh1"], np.float32)
    b1 = np.asarray(inputs["b_ih1"], np.float32) + np.asarray(inputs["b_hh1"], np.float32)
    proj_w = np.asarray(inputs["proj_w"], np.float32)
    proj_b = np.asarray(inputs["proj_b"], np.float32)

    TB = T * B
    in0 = D + C
    xs = np.concatenate([np.zeros((B, D, 1), np.float32), x[:, :, : T - 1]], axis=2)
    xin = np.concatenate([xs, cond], axis=1)
    xin_pad = np.zeros((B, KIN * P, T), np.float32)
    xin_pad[:, :in0] = xin
    xin_pad[:, in0] = 1.0
    xinT = np.ascontiguousarray(xin_pad.transpose(1, 2, 0)).reshape(KIN * P, TB)
    xinT = np.ascontiguousarray(
        xinT.reshape(KIN, P, TB).transpose(1, 0, 2)
    ).astype(NP_BF16)

    w_ih0_pad = np.zeros((4 * H, KIN * P), np.float32)
    w_ih0_pad[:, :in0] = w_ih0
    w_ih0_pad[:, in0] = b0

    ident = np.eye(P, dtype=NP_BF16)

    in_maps = []
    for k in range(NC):
        r = _gate_rows(k)
        w0T = _lhsT_blocks(_scale_g(w_ih0_pad[r]), KIN).astype(NP_BF16)
        wh0T = _lhsT_blocks(_scale_g(w_hh0[r]), KH).astype(NP_BF16)
        w1_ext = np.zeros((MT * P, (KH + 1) * P), np.float32)
        w1_ext[:, : KH * P] = w_ih1[r]
        w1_ext[:, KH * P] = b1[r]
        w1T = _lhsT_blocks(_scale_g(w1_ext), KH + 1).astype(NP_BF16)
        wh1T = _lhsT_blocks(_scale_g(w_hh1[r]), KH).astype(NP_BF16)
        pjT = np.zeros((P, KH + 1, PJP), np.float32)
        rows = np.arange(k * PJ, (k + 1) * PJ)
        for kk in range(KH):
            pjT[:, kk, :PJ] = proj_w[rows, kk * P : (kk + 1) * P].T
        pjT[0, KH, :PJ] = proj_b[rows]
        in_maps.append(
            {
                "xinT": xinT,
                "w0T": w0T,
                "wh0T": wh0T,
                "w1T": w1T,
                "wh1T": wh1T,
                "pjT": pjT.astype(NP_BF16),
                "ident": ident,
            }
        )
    return in_maps


def assemble(results, x_lengths, T):
    y = np.concatenate([r["y_out"][:PJ] for r in results], axis=0)
    y = y.reshape(D, T, B).transpose(2, 0, 1)
    lens = np.asarray(x_lengths).astype(np.int64)
    mask = (np.arange(T)[None, :] < lens[:, None]).astype(np.float32)
    return np.ascontiguousarray(y * mask[:, None, :])


_NC_CACHE = {}


def run(inputs, T=T_FULL, trace=False, **kw):
    if T not in _NC_CACHE:
        _NC_CACHE[T] = build_nc(T)
    nc = _NC_CACHE[T]
    in_maps = prep_inputs(inputs, T)
    res = bass_utils.run_bass_kernel_spmd(
        nc, in_maps, core_ids=list(range(NC)), trace=trace, **kw
    )
    out = assemble(res.results, inputs["x_lengths"], T)
    return out, res


def kernel(**inputs) -> np.ndarray:
    out, _ = run(inputs, T=T_FULL)
    return out


# revision 4
# speedup vs baseline: 3.4091x; 1.0625x over previous
"""Trainium2 Bass kernel for the 2-layer autoregressive LSTM (teacher-forced).

v2 -> v3 changes, driven by cost-model + HW ablation findings:
  - Sigmoid-only activations: tanh(x) = 2*sigmoid(2x) - 1, with the 2x folded
    into the g-gate weight rows host-side.  One activation per cell covers all
    four gates; no ACT table reloads (1.28us each) ever.
  - Combined history: one SBUF tile holds [h0(t), h1(t-LAG)] per ring slot
    (slot = t mod 2L, LAG = 2L), so ONE history DMA per step instead of two.
  - Chunked GEMMs read strided 3D rhs views of the combined history.
  - Element-wise cell trimmed to 2 ACT + 9 DVE ops via two-stage
    tensor_scalar affines.
"""

import numpy as np

import concourse.bass as bass
import concourse.bacc as bacc
import concourse.tile as tile
from concourse import bass_utils, mybir

AF = mybir.ActivationFunctionType
ALU = mybir.AluOpType

# Problem constants
B, D, T_FULL, C, H = 32, 80, 1000, 512, 1024
ZONEOUT = 0.1

# Kernel layout constants
P = 128
NC = 8
HU = H // NC
MT = 4
KIN = 5
KH = H // P      # 8
L = 16
LAG = 2 * L      # layer-1 lags a full ring behind layer 0
PJ = D // NC
PJP = 16

BF16 = mybir.dt.bfloat16
F32 = mybir.dt.float32
NP_BF16 = mybir.dt.np(BF16)

RG = [list(range(NC))]


def _chunks(T):
    n = (T + L - 1) // L
    return [(c, min(L, T - c * L)) for c in range(n)]


def build_nc(T):
    TB = T * B
    nc = bacc.Bacc(
        "TRN2",
        target_bir_lowering=False,
        debug=False,
        enable_asserts=False,
        num_devices=NC,
    )

    xinT_d = nc.dram_tensor("xinT", [P, KIN, TB], BF16, kind="ExternalInput")
    w0T_d = nc.dram_tensor("w0T", [P, KIN, MT, P], BF16, kind="ExternalInput")
    wh0T_d = nc.dram_tensor("wh0T", [P, KH, MT, P], BF16, kind="ExternalInput")
    w1T_d = nc.dram_tensor("w1T", [P, KH + 1, MT, P], BF16, kind="ExternalInput")
    wh1T_d = nc.dram_tensor("wh1T", [P, KH, MT, P], BF16, kind="ExternalInput")
    pjT_d = nc.dram_tensor("pjT", [P, KH + 1, PJP], BF16, kind="ExternalInput")
    id_d = nc.dram_tensor("ident", [P, P], BF16, kind="ExternalInput")
    y_d = nc.dram_tensor("y_out", [PJP, TB], F32, kind="ExternalOutput")

    ch = _chunks(T)
    # spread each chunk GEMM over 4 consecutive steps (one m-tile per step)
    # so the work fills the exchange-latency window instead of bursting.
    u0_at = {}
    u1_at = {}
    for c, lc in ch:
        if c >= 1:
            for m in range(MT):
                u0_at.setdefault((c - 1) * L + m, []).append((c, m))
        else:
            pass
        for m in range(MT):
            u1_at.setdefault(c * L + lc + m, []).append((c, m))
    pj_at = {c * L + lc + LAG: c for c, lc in ch}

    with tile.TileContext(nc) as tc:
        with (
            tc.tile_pool(name="const", bufs=1) as cp,
            tc.tile_pool(name="dram", bufs=1, space="DRAM") as dp,
            tc.tile_pool(name="psc", bufs=1, space="PSUM") as pscp,
            tc.tile_pool(name="psu", bufs=1, space="PSUM") as psup,
        ):
            w0_sb = cp.tile([P, KIN, MT, P], BF16)
            wh0_sb = cp.tile([P, KH, MT, P], BF16)
            w1_sb = cp.tile([P, KH + 1, MT, P], BF16)
            wh1_sb = cp.tile([P, KH, MT, P], BF16)
            pj_sb = cp.tile([P, KH + 1, PJP], BF16)
            id_sb = cp.tile([P, P], BF16)
            # combined ring history: slot s holds [h0(s), h1(s-LAG)] slices
            hist = cp.tile([P, KH, 2 * L, 2 * B], BF16)
            ones_sb = cp.tile([P, L, B], BF16)
            U0_sb = cp.tile([P, 2, MT, L * B], BF16)
            U1_sb = cp.tile([P, 2, MT, L * B], BF16)
            xin_t = cp.tile([P, 2, KIN, L * B], BF16)
            y_t = cp.tile([PJP, 2, L * B], F32)

            c0_t = cp.tile([P, B], F32)
            c1_t = cp.tile([P, B], F32)

            send_t = cp.tile([P, 2, 2 * B], BF16)
            S_t = cp.tile([P, 2, 2, MT * B], F32)
            tg_t = cp.tile([P, 2, 2, B], F32)
            cn_t = cp.tile([P, 2, 2, B], F32)
            c01_t = cp.tile([P, 2, 2, B], F32)
            h01_t = cp.tile([P, 2, 2, B], F32)
            sc_t = cp.tile([P, 2, 2, B], F32)
            Hz_t = cp.tile([P, 2, 2, B], F32)
            R_t = cp.tile([P, 2, 2, B], F32)
            Pi_t = cp.tile([P, 2, 2, B], F32)

            ps_cell = pscp.tile([P, 4, MT * B], F32)
            ps_u = psup.tile([P, 2, L * B], F32)
            ps_pj = psup.tile([PJP, 2, L * B], F32)

            agi0 = dp.tile([P, 2 * B], BF16, tag="agi0")
            agi1 = dp.tile([P, 2 * B], BF16, tag="agi1")
            ago0 = dp.tile([NC * P, 2 * B], BF16, tag="ago0")
            ago1 = dp.tile([NC * P, 2 * B], BF16, tag="ago1")
            agi = [agi0, agi1]
            ago = [ago0, ago1]

            nc.sync.dma_start(w0_sb[:], w0T_d[:])
            nc.sync.dma_start(wh0_sb[:], wh0T_d[:])
            nc.sync.dma_start(w1_sb[:], w1T_d[:])
            nc.sync.dma_start(wh1_sb[:], wh1T_d[:])
            nc.sync.dma_start(pj_sb[:], pjT_d[:])
            nc.sync.dma_start(id_sb[:], id_d[:])

            nc.vector.memset(hist[:], 0.0)
            nc.vector.memset(ones_sb[:], 1.0)
            nc.vector.memset(c0_t[:], 0.0)
            nc.vector.memset(c1_t[:], 0.0)
            nc.vector.memset(send_t[:], 0.0)

            def h0_rhs(k, c, lc):
                # h0 over chunk-c timesteps: slots c*L .. c*L+lc-1, layer-0 half
                half = (c * L) % (2 * L)
                return hist[:, k, half : half + lc, 0:B]

            def h1_rhs(k, c, lc):
                # h1(tau) lives in slot tau+LAG = tau (mod 2L), layer-1 half
                half = (c * L) % (2 * L)
                return hist[:, k, half : half + lc, B : 2 * B]

            def emit_u0(c, m):
                lc = ch[c][1]
                nco = lc * B
                xt = xin_t[:, c % 2, :, :]
                if m == 0:
                    nc.sync.dma_start(
                        xt[:, :, :nco], xinT_d[:, :, c * L * B : c * L * B + nco]
                    )
                pt = ps_u[:, c % 2, :]
                for k in range(KIN):
                    nc.tensor.matmul(
                        pt[:, :nco],
                        w0_sb[:, k, m, :],
                        xt[:, k, :nco],
                        start=(k == 0),
                        stop=(k == KIN - 1),
                    )
                nc.vector.tensor_copy(U0_sb[:, c % 2, m, :nco], pt[:, :nco])

            def emit_u1(c, m):
                lc = ch[c][1]
                nco = lc * B
                pt = ps_u[:, c % 2, :]
                for k in range(KH + 1):
                    if k < KH:
                        rhs = h0_rhs(k, c, lc)
                    else:
                        rhs = ones_sb[:, 0:lc, :]
                    nc.tensor.matmul(
                        pt[:, :nco],
                        w1_sb[:, k, m, :],
                        rhs,
                        start=(k == 0),
                        stop=(k == KH),
                    )
                nc.vector.tensor_copy(U1_sb[:, c % 2, m, :nco], pt[:, :nco])

            def emit_proj(c):
                lc = ch[c][1]
                nco = lc * B
                pt = ps_pj[:, c % 2, :]
                for k in range(KH + 1):
                    if k < KH:
                        rhs = h1_rhs(k, c, lc)
                    else:
                        rhs = ones_sb[:, 0:lc, :]
                    nc.tensor.matmul(
                        pt[:, :nco],
                        pj_sb[:, k, :],
                        rhs,
                        start=(k == 0),
                        stop=(k == KH),
                    )
                yt = y_t[:, c % 2, :]
                nc.scalar.copy(yt[:, :nco], pt[:, :nco])
                nc.sync.dma_start(y_d[:, c * L * B : c * L * B + nco], yt[:, :nco])

            def cell(ell, t, par):
                off = 0 if ell == 0 else B
                W = wh0_sb if ell == 0 else wh1_sb
                U = U0_sb if ell == 0 else U1_sb
                cst = c0_t if ell == 0 else c1_t
                ps = ps_cell[:, ell * 2 + par, :]
                slot = (t - 1) % (2 * L)
                ci, si = t // L, t % L
                for m in range(MT):
                    o = ps[:, m * B : (m + 1) * B]
                    for k in range(KH):
                        nc.tensor.matmul(
                            o, W[:, k, m, :], hist[:, k, slot, off : off + B],
                            start=(k == 0), stop=False,
                        )
                    nc.tensor.matmul(
                        o, id_sb[:],
                        U[:, ci % 2, m, si * B : (si + 1) * B],
                        start=False, stop=True,
                    )
                S = S_t[:, par, ell, :]
                tg = tg_t[:, par, ell, :]
                cn = cn_t[:, par, ell, :]
                c01 = c01_t[:, par, ell, :]
                h01 = h01_t[:, par, ell, :]
                sc = sc_t[:, par, ell, :]
                Hz = Hz_t[:, par, ell, :]
                R = R_t[:, par, ell, :]
                Pi = Pi_t[:, par, ell, :]
                # gates: [i, f, o, 2g] -> sigmoid of all four at once
                nc.scalar.activation(S[:], ps[:], AF.Sigmoid)
                # tanh(g) = 2*sigmoid(2g) - 1
                nc.vector.tensor_scalar(
                    tg[:], S[:, 3 * B : 4 * B], 2.0, 1.0,
                    op0=ALU.mult, op1=ALU.subtract,
                )
                nc.vector.tensor_mul(Pi[:], S[:, 0:B], tg[:])
                nc.vector.tensor_mul(R[:], S[:, B : 2 * B], cst[:])
                nc.vector.tensor_add(cn[:], R[:], Pi[:])
                nc.vector.tensor_scalar_mul(c01[:], cst[:], ZONEOUT)
                nc.vector.scalar_tensor_tensor(
                    cst[:], cn[:], 1.0 - ZONEOUT, c01[:], op0=ALU.mult, op1=ALU.add
                )
                # tanh(c_new) = 2*sigmoid(2*c_new) - 1, folded with 0.9:
                # 0.9*tanh(cn) = 1.8*sigmoid(2*cn) - 0.9
                nc.scalar.activation(sc[:], cn[:], AF.Sigmoid, scale=2.0)
                nc.vector.tensor_scalar(
                    Hz[:], sc[:], 2.0 * (1.0 - ZONEOUT), 1.0 - ZONEOUT,
                    op0=ALU.mult, op1=ALU.subtract,
                )
                nc.vector.tensor_scalar_mul(
                    h01[:], send_t[:, 1 - par, off : off + B], ZONEOUT
                )
                nc.vector.tensor_mul(Hz[:], Hz[:], S[:, 2 * B : 3 * B])
                nc.vector.tensor_add(send_t[:, par, off : off + B], Hz[:], h01[:])

            for m0 in range(MT):
                emit_u0(0, m0)

            for t in range(T + LAG):
                par = t % 2
                if t < T:
                    cell(0, t, par)
                tau = t - LAG
                if tau >= 0:
                    cell(1, tau, par)

                nc.gpsimd.dma_start(agi[par][:], send_t[:, par, :])
                nc.gpsimd.collective_compute(
                    "AllGather",
                    ALU.bypass,
                    replica_groups=RG,
                    ins=[agi[par].opt()],
                    outs=[ago[par].opt()],
                )
                agov = ago[par][:].rearrange("(k p) b -> p k b", p=P)
                nc.gpsimd.dma_start(hist[:, :, t % (2 * L), :], agov)

                for (cc_, mm_) in u0_at.get(t, ()):
                    emit_u0(cc_, mm_)
                for (cc_, mm_) in u1_at.get(t, ()):
                    emit_u1(cc_, mm_)
                if t in pj_at:
                    emit_proj(pj_at[t])

            for t_late in sorted(pj_at):
                if t_late >= T + LAG:
                    emit_proj(pj_at[t_late])

    nc.compile()
    return nc


# ---------------- host-side data prep ----------------

def _gate_rows(k):
    u = np.arange(k * HU, (k + 1) * HU)
    return np.concatenate([u, H + u, 3 * H + u, 2 * H + u])  # i, f, o, g


def _scale_g(w):
    """Scale the g-gate rows (last P block) by 2 for the sigmoid-only trick."""
    w = w.copy()
    w[3 * P :] *= 2.0
    return w


def _lhsT_blocks(w, nk, mt=MT):
    a = w.reshape(mt, P, nk, P)
    return np.ascontiguousarray(a.transpose(3, 2, 0, 1))


def prep_inputs(inputs, T):
    x = np.asarray(inputs["x"], np.float32)[:, :, :T]
    cond = np.asarray(inputs["cond"], np.float32)[:, :, :T]
    w_ih0 = np.asarray(inputs["w_ih0"], np.float32)
    w_hh0 = np.asarray(inputs["w_hh0"], np.float32)
    b0 = np.asarray(inputs["b_ih0"], np.float32) + np.asarray(inputs["b_hh0"], np.float32)
    w_ih1 = np.asarray(inputs["w_ih1"], np.float32)
    w_hh1 = np.asarray(inputs["w_hh1"], np.float32)
    b1 = np.asarray(inputs["b_ih1"], np.float32) + np.asarray(inputs["b_hh1"], np.float32)
    proj_w = np.asarray(inputs["proj_w"], np.float32)
    proj_b = np.asarray(inputs["proj_b"], np.float32)

    TB = T * B
    in0 = D + C
    xs = np.concatenate([np.zeros((B, D, 1), np.float32), x[:, :, : T - 1]], axis=2)
    xin = np.concatenate([xs, cond], axis=1)
    xin_pad = np.zeros((B, KIN * P, T), np.float32)
    xin_pad[:, :in0] = xin
    xin_pad[:, in0] = 1.0
    xinT = np.ascontiguousarray(xin_pad.transpose(1, 2, 0)).reshape(KIN * P, TB)
    xinT = np.ascontiguousarray(
        xinT.reshape(KIN, P, TB).transpose(1, 0, 2)
    ).astype(NP_BF16)

    w_ih0_pad = np.zeros((4 * H, KIN * P), np.float32)
    w_ih0_pad[:, :in0] = w_ih0
    w_ih0_pad[:, in0] = b0

    ident = np.eye(P, dtype=NP_BF16)

    in_maps = []
    for k in range(NC):
        r = _gate_rows(k)
        w0T = _lhsT_blocks(_scale_g(w_ih0_pad[r]), KIN).astype(NP_BF16)
        wh0T = _lhsT_blocks(_scale_g(w_hh0[r]), KH).astype(NP_BF16)
        w1_ext = np.zeros((MT * P, (KH + 1) * P), np.float32)
        w1_ext[:, : KH * P] = w_ih1[r]
        w1_ext[:, KH * P] = b1[r]
        w1T = _lhsT_blocks(_scale_g(w1_ext), KH + 1).astype(NP_BF16)
        wh1T = _lhsT_blocks(_scale_g(w_hh1[r]), KH).astype(NP_BF16)
        pjT = np.zeros((P, KH + 1, PJP), np.float32)
        rows = np.arange(k * PJ, (k + 1) * PJ)
        for kk in range(KH):
            pjT[:, kk, :PJ] = proj_w[rows, kk * P : (kk + 1) * P].T
        pjT[0, KH, :PJ] = proj_b[rows]
        in_maps.append(
            {
                "xinT": xinT,
                "w0T": w0T,
                "wh0T": wh0T,
                "w1T": w1T,
                "wh1T": wh1T,
                "pjT": pjT.astype(NP_BF16),
                "ident": ident,
            }
        )
    return in_maps


def assemble(results, x_lengths, T):
    y = np.concatenate([r["y_out"][:PJ] for r in results], axis=0)
    y = y.reshape(D, T, B).transpose(2, 0, 1)
    lens = np.asarray(x_lengths).astype(np.int64)
    mask = (np.arange(T)[None, :] < lens[:, None]).astype(np.float32)
    return np.ascontiguousarray(y * mask[:, None, :])


_NC_CACHE = {}


def run(inputs, T=T_FULL, trace=False, **kw):
    if T not in _NC_CACHE:
        _NC_CACHE[T] = build_nc(T)
    nc = _NC_CACHE[T]
    in_maps = prep_inputs(inputs, T)
    res = bass_utils.run_bass_kernel_spmd(
        nc, in_maps, core_ids=list(range(NC)), trace=trace, **kw
    )
    out = assemble(res.results, inputs["x_lengths"], T)
    return out, res


def kernel(**inputs) -> np.ndarray:
    out, _ = run(inputs, T=T_FULL)
    return out
